# revision 23
# baseline (speedup 1.0000x reference)
"""Trainium2 Bass kernel for nn_Attention_86268713108190.

7 independent attention "bands" over batch 8, n=512, d=512, 8 heads,
shared Wqkv/Wout. Sharding: data-parallel over batch — core c handles
batch index c (7 band-samples of [512, 512] each).

Per-core dataflow (per band). The key idea vs the previous version is a
ROW-MAJOR AV: PE matmul time is (output free size) x cycles/row
regardless of how many partitions are written, so the old
OT-layout AV (psum [65, 512] per head, free=512) paid 2x the PE rows
AND left the softmax denominator on the partition axis, forcing a
~190us/7-band cross-engine partition-broadcast tail. Instead:

  1. qkT = Wqkv @ x^T chunks (f32r)                       [8*128, n]
  2. v = x @ Wv^T -> row-major v_aug [j, h, 65] in bf16 with a ones
     column per head (the softmax denominator falls out of AV free).
  3. per head pair g: S^T[j,i] = K_h Q_h^T (f32r, K=64, two heads
     packed via tile_position), expS^T = exp(SCALE*S^T) on ACT -> bf16.
  4. AV row-major: psum[i, h01, 65] += es^T[j,i-chunk]^T @ v_aug[j,h]
     (bf16: f32r would hit the 4x small-free penalty at free=65).
     Column 64 of each head slot = denominator d_i, PER PARTITION, so
     normalization is a DVE reciprocal [128,2] + tensor_scalar_mul with
     a [P,1] scalar operand — no partition broadcast at all.
     Head pairs are emitted interleaved (S0 S1 AV0 S2 AV1 S3 AV2 AV3)
     so AV(g) never waits on ACT's exp stream.
  5. O row-major -> O^T via 16 PE transposes (bf16 identity,
     128 rows each), then out = O @ Wout^T + bias (bf16).
"""

import contextlib
import sys

if '/opt/trn_rl_repo' not in sys.path:
    sys.path.insert(0, '/opt/trn_rl_repo')

import numpy as np

P = 128
NSEQ = 512
D = 512
H = 8
DH = 64
NBANDS = 7
NCORES = 8
SCALE = D ** -0.5

_cached = None


def _emit_qkv(ctx, xt):
    """QKV projections for one band; returns (qk_sb, v_aug)."""
    nc, f32, f32r, bf16 = ctx["nc"], ctx["f32"], ctx["f32r"], ctx["bf16"]
    wq_sb = ctx["wq_sb"]
    pl = ctx["pools"]

    # q,k -> qkvT layout; q and k in SEPARATE tiles: the S matmuls read
    # lhsT from k and rhs from q, and same-tile operands measurably
    # slow the PE (SBUF port contention)
    q_sb = pl["qk"].tile([P, 4, NSEQ], ctx["qkdt"], tag="qk", name="q_sb")
    k_sb = pl["qk"].tile([P, 4, NSEQ], ctx["qkdt"], tag="qk", name="k_sb")
    for et in (0, 4, 1, 5, 2, 6, 3, 7):
        ps = pl["psproj"].tile([P, NSEQ], f32, tag="psproj", name="ps_qk")
        for kt in range(4):
            nc.tensor.matmul(
                ps[:], wq_sb[:, kt, et * P:(et + 1) * P], xt[:, kt, :],
                start=(kt == 0), stop=(kt == 3))
        dst = q_sb if et < 4 else k_sb
        nc.vector.tensor_copy(dst[:, et % 4, :], ps[:])

    # V projection -> row-major bf16 v_aug with ones column
    v_aug = pl["v"].tile([P, 4, H, DH + 1], bf16, tag="vaug", name="v_aug")
    for nt in range(4):
        ps = pl["psproj"].tile([P, NSEQ], f32, tag="psproj", name="ps_v")
        for kt in range(4):
            nc.tensor.matmul(
                ps[:], xt[:, kt, nt * P:(nt + 1) * P],
                wq_sb[:, kt, 2 * D:3 * D],
                start=(kt == 0), stop=(kt == 3))
        nc.vector.tensor_copy(
            v_aug[:, nt, :, 0:DH],
            ps[:].rearrange("p (h dh) -> p h dh", h=H))
        nc.vector.memset(v_aug[:, nt, :, DH:DH + 1], 1.0)
    return (q_sb, k_sb), v_aug


def _emit_attn(ctx, qk, v_aug):
    """S -> exp -> row-major AV -> per-partition normalize. Returns the
    four row-major O tiles (one per i-chunk)."""
    q_sb, k_sb = qk
    nc, f32, bf16, Exp = ctx["nc"], ctx["f32"], ctx["bf16"], ctx["Exp"]
    pl = ctx["pools"]

    es_store = {}
    o_sb = {}

    def s_phase(g):
        es_list = []
        for jt in range(4):
            # both head-halves in one 2-bank psum tile -> single exp
            ps_s = pl["pss"].tile([P, 2, NSEQ], f32, tag="pss", name="ps_s")
            nc.tensor.matmul(
                ps_s[:, 0, :],
                k_sb[0:DH, g, jt * P:(jt + 1) * P],
                q_sb[0:DH, g, :], start=True, stop=True)
            nc.tensor.matmul(
                ps_s[:, 1, :],
                k_sb[DH:P, g, jt * P:(jt + 1) * P],
                q_sb[DH:P, g, :], start=True, stop=True,
                tile_position=(DH, 0))
            es = pl["es"].tile([P, 2, NSEQ], bf16, tag="es", name="es")
            nc.scalar.activation(es[:], ps_s[:], Exp, scale=SCALE)
            es_list.append(es)
        es_store[g] = es_list

    def av_phase(g):
        es_list = es_store.pop(g)
        for ic in range(4):
            if g == 0:
                o_sb[ic] = pl["o"].tile([P, D], bf16, tag="osb",
                                        name=f"osb{ic}")
            if ctx["ablate"] == "no_av":
                if g == 0:
                    nc.vector.tensor_copy(o_sb[ic][:], es_list[0][:, 0, :])
                continue
            ps_av = pl["psav"].tile([P, 2, DH + 1], f32, tag="psav",
                                    name="ps_av")
            for h01 in range(2):
                for jt in range(4):
                    nc.tensor.matmul(
                        ps_av[:, h01, :],
                        es_list[jt][:, h01, ic * P:(ic + 1) * P],
                        v_aug[:, jt, 2 * g + h01, :],
                        start=(jt == 0), stop=(jt == 3))
            rcp = pl["r"].tile([P, 2], f32, tag="rcp", name="rcp")
            nc.vector.reciprocal(rcp[:], ps_av[:, :, DH])
            for h01 in range(2):
                hd = 2 * g + h01
                nc.vector.tensor_scalar_mul(
                    o_sb[ic][:, hd * DH:(hd + 1) * DH],
                    ps_av[:, h01, 0:DH],
                    rcp[:, h01:h01 + 1])

    s_phase(0)
    s_phase(1)
    av_phase(0)
    s_phase(2)
    av_phase(1)
    s_phase(3)
    av_phase(2)
    av_phase(3)
    return o_sb


def _emit_transposes(ctx, o_sb):
    """O row-major -> O^T via PE transposes; returns ot_sb."""
    nc, bf16 = ctx["nc"], ctx["bf16"]
    ident = ctx["ident"]
    pl = ctx["pools"]

    ot_sb = pl["ot"].tile([P, 4, NSEQ], bf16, tag="ot", name="ot_sb")
    for ec in range(4):
        ps_t = pl["psav"].tile([P, NSEQ], bf16, tag="psav", name="ps_t")
        for ic in range(4):
            nc.tensor.transpose(
                ps_t[:, ic * P:(ic + 1) * P],
                o_sb[ic][:, ec * P:(ec + 1) * P],
                ident[:])
        nc.vector.tensor_copy(ot_sb[:, ec, :], ps_t[:])
    return ot_sb


def _emit_outproj(ctx, s, ot_sb):
    nc, f32 = ctx["nc"], ctx["f32"]
    wo_sb, bias_sb, out = ctx["wo_sb"], ctx["bias_sb"], ctx["out"]
    pl = ctx["pools"]

    for nt in range(4):
        ps = pl["psproj"].tile([P, NSEQ], f32, tag="psproj", name="ps_o")
        for kt in range(4):
            nc.tensor.matmul(
                ps[:], ot_sb[:, kt, nt * P:(nt + 1) * P], wo_sb[:, kt, :],
                start=(kt == 0), stop=(kt == 3))
        ob = pl["ob"].tile([P, D], f32, tag="ob", name="ob")
        nc.vector.tensor_add(ob[:], ps[:], bias_sb[:])
        nc.sync.dma_start(
            out[s].rearrange("(no ni) e -> ni no e", ni=P)[:, nt, :], ob[:])


def build_kernel(nbands=NBANDS, repeat=1, ablate="", sdt="bf16",
                 qkvdt="bf16"):
    import concourse.mybir as mybir
    import concourse.tile as tile
    from concourse import bacc

    f32 = mybir.dt.float32
    f32r = mybir.dt.float32r
    bf16 = mybir.dt.bfloat16
    Exp = mybir.ActivationFunctionType.Exp

    nc = bacc.Bacc("TRN2", target_bir_lowering=False, debug=False,
                   num_devices=NCORES)

    xdt = f32r if qkvdt == "f32r" else bf16
    xT = nc.dram_tensor("xT", [nbands, D, NSEQ], xdt, kind="ExternalInput").ap()
    wqkvT = nc.dram_tensor("wqkvT", [D, 3 * D], xdt, kind="ExternalInput").ap()
    woutT = nc.dram_tensor("woutT", [D, D], bf16, kind="ExternalInput").ap()
    biasb = nc.dram_tensor("biasb", [P, D], f32, kind="ExternalInput").ap()
    identT = nc.dram_tensor("identT", [P, P], bf16, kind="ExternalInput").ap()
    out = nc.dram_tensor("out", [nbands, NSEQ, D], f32, kind="ExternalOutput").ap()

    with tile.TileContext(nc) as tc:
        with (
            tc.tile_pool(name="weights", bufs=1) as wpool,
            tc.tile_pool(name="x", bufs=3) as xpool,
            tc.tile_pool(name="qk", bufs=2) as qkpool,
            tc.tile_pool(name="v", bufs=2) as vpool,
            tc.tile_pool(name="o", bufs=8) as opool,
            tc.tile_pool(name="ot", bufs=2) as otpool,
            tc.tile_pool(name="es", bufs=8) as spool,
            tc.tile_pool(name="r", bufs=4) as rpool,
            tc.tile_pool(name="ob", bufs=3) as outpool,
            tc.tile_pool(name="psproj", bufs=2, space="PSUM") as psproj,
            tc.tile_pool(name="pss", bufs=2, space="PSUM") as pss,
            tc.tile_pool(name="psav", bufs=2, space="PSUM") as psav,
        ):
            # weights: split wq by k-chunk so the first matmuls can start
            # as soon as their chunk lands
            wq_sb = wpool.tile([P, 4, 3 * D], xdt)
            wo_sb = wpool.tile([P, 4, D], bf16)
            bias_sb = wpool.tile([P, D], f32)
            ident = wpool.tile([P, P], bf16)
            wq_r = wqkvT.rearrange("(ko ki) e -> ki ko e", ki=P)
            for kt in range(4):
                nc.sync.dma_start(wq_sb[:, kt, :], wq_r[:, kt, :])
            nc.sync.dma_start(wo_sb[:], woutT.rearrange("(ko ki) e -> ki ko e", ki=P))
            nc.sync.dma_start(bias_sb[:], biasb[:])
            nc.sync.dma_start(ident[:], identT[:])

            ctx = {
                "nc": nc, "ablate": ablate,
                "qkdt": f32r if sdt == "f32r" else bf16,
                "f32": f32, "f32r": f32r, "bf16": bf16, "Exp": Exp,
                "wq_sb": wq_sb, "wo_sb": wo_sb, "bias_sb": bias_sb,
                "ident": ident,
                "out": out,
                "pools": {
                    "qk": qkpool, "v": vpool, "o": opool, "ot": otpool,
                    "es": spool, "r": rpool, "ob": outpool,
                    "psproj": psproj, "pss": pss, "psav": psav,
                },
            }

            def load_x(s):
                xt = xpool.tile([P, 4, NSEQ], xdt, tag="xt")
                nc.sync.dma_start(
                    xt[:], xT[s].rearrange("(ko ki) n -> ki ko n", ki=P))
                return xt

            rep_ctx = (tc.For_i(0, repeat, 1,
                                hint_engines=(mybir.EngineType.PE,
                                              mybir.EngineType.Activation,
                                              mybir.EngineType.DVE))
                       if repeat > 1 else contextlib.nullcontext())
            with rep_ctx:
                # software pipeline: attn(s) -> transposes(s) -> QKV(s+1)
                # -> out-proj(s), so transpose copies and out-proj deps
                # hide under next-band QKV matmuls.
                xt = load_x(0)
                qkv = _emit_qkv(ctx, xt)
                for s in range(nbands):
                    if s + 1 < nbands:
                        xt_next = load_x(s + 1)
                    if ctx["ablate"] == "no_attn":
                        o_sb = {}
                        for ic in range(4):
                            o_sb[ic] = opool.tile([P, D], bf16, tag="osb",
                                                  name=f"osb{ic}")
                            nc.vector.tensor_copy(
                                o_sb[ic][:].rearrange(
                                    "p (h dh) -> p h dh", h=H),
                                qkv[1][:, ic, :, 0:DH])
                    else:
                        o_sb = _emit_attn(ctx, *qkv)
                    ot_sb = _emit_transposes(ctx, o_sb)
                    if s + 1 < nbands:
                        qkv = _emit_qkv(ctx, xt_next)
                    _emit_outproj(ctx, s, ot_sb)

    nc.compile()
    return nc


def _get_nc():
    global _cached
    if _cached is None:
        _cached = build_kernel()
    return _cached


def make_in_maps(x, x_delta, x_theta, x_alpha, x_beta, x_gamma, x_upper,
                 Wqkv, Wout, bout, qkvdt="bf16"):
    import ml_dtypes
    bf16 = ml_dtypes.bfloat16
    xdt = np.float32 if qkvdt == "f32r" else bf16
    xs = np.stack([np.asarray(a, dtype=np.float32) for a in
                   (x, x_delta, x_theta, x_alpha, x_beta, x_gamma, x_upper)],
                  axis=0)  # [7, b, n, d]
    xsT = np.ascontiguousarray(xs.transpose(1, 0, 3, 2).astype(xdt))
    wqkvT = np.ascontiguousarray(np.asarray(Wqkv, np.float32).T.astype(xdt))
    woutT = np.ascontiguousarray(np.asarray(Wout, np.float32).T.astype(bf16))
    biasb = np.ascontiguousarray(
        np.broadcast_to(np.asarray(bout, np.float32)[None, :], (P, D)))
    identT = np.eye(P, dtype=bf16)
    return [
        {"xT": xsT[c], "wqkvT": wqkvT, "woutT": woutT, "biasb": biasb,
         "identT": identT}
        for c in range(NCORES)
    ]


def kernel(x, x_delta, x_theta, x_alpha, x_beta, x_gamma, x_upper,
           Wqkv, Wout, bout):
    from concourse.bass_utils import run_bass_kernel_spmd

    nc = _get_nc()
    in_maps = make_in_maps(x, x_delta, x_theta, x_alpha, x_beta, x_gamma,
                           x_upper, Wqkv, Wout, bout)
    res = run_bass_kernel_spmd(nc, in_maps, core_ids=list(range(NCORES)))
    full = np.empty((NBANDS, NCORES, NSEQ, D), dtype=np.float32)
    for c in range(NCORES):
        full[:, c] = res.results[c]["out"]
    return tuple(full[i] for i in range(NBANDS))


# revision 25
# speedup vs baseline: 1.1061x; 1.1061x over previous
"""Trainium2 Bass kernel for nn_Attention_86268713108190.

7 independent attention "bands" over batch 8, n=512, d=512, 8 heads,
shared Wqkv/Wout. Sharding: data-parallel over batch — core c handles
batch index c (7 band-samples of [512, 512] each).

Per-core dataflow (per band), all matmul operands bf16 by default
(whole-output rel err ~4.6e-3 vs the 2e-2 gate; f32r is no faster per
PE row and costs 2x operand bandwidth — measured -20 to -35us/iter for
bf16). The key idea vs the original baseline is a ROW-MAJOR AV: PE
matmul time is (output free size) x cycles/row regardless of how many
partitions are written, so the old OT-layout AV (psum [65, 512] per
head, free=512) paid 2x the PE rows AND left the softmax denominator
on the partition axis, forcing a ~190us/7-band cross-engine
partition-broadcast tail. Instead:

  1. qT, kT = Wqkv @ x^T chunks, into SEPARATE q/k SBUF tiles (the S
     matmuls read lhsT from k and rhs from q; same-tile PE operands
     measurably stall on SBUF port contention).
  2. v = x @ Wv^T -> row-major v_aug [j, h, 65] in bf16 with a ones
     column per head (the softmax denominator falls out of AV free).
  3. per head pair g: S^T[j,i] = K_h Q_h^T (K=64, two heads packed in
     one 2-bank psum via tile_position), expS^T = exp(SCALE*S^T) as a
     single fused [128,2,512] ACT instruction -> bf16.
  4. AV row-major: psum[i, h01, 65] += es^T[j,i-chunk]^T @ v_aug[j,h].
     Column 64 of each head slot = denominator d_i, PER PARTITION, so
     normalization is a DVE reciprocal [128,2] + tensor_scalar_mul with
     a [P,1] scalar operand — no partition broadcast at all.
     Head pairs are emitted interleaved (S0 S1 AV0 S2 AV1 S3 AV2 AV3)
     so AV(g) never waits on ACT's exp stream. (Fusing ALL S matmuls
     into the QKV phase simmed faster but measured ~30us/iter SLOWER
     on HW — batched S phases keep the PE tile-config stable.)
  5. O row-major -> O^T via 16 PE transposes (bf16 identity,
     128 rows each), then out = O @ Wout^T + bias (bf16).

Cross-band software pipeline: attn(s) -> transposes(s) -> QKV(s+1) ->
out-proj(s), hiding transpose copies and out-proj deps under next-band
QKV matmuls. PSUM: psproj 2 + pss 2x2 + psav 2 = 8 banks exactly.
Measured (R=1 vs R=201 For_i repeat differencing, 8 cores): ~240-266us
per 7-band iteration vs 551.7us baseline; rel err 4.6e-3.
"""

import contextlib
import sys

if '/opt/trn_rl_repo' not in sys.path:
    sys.path.insert(0, '/opt/trn_rl_repo')

import numpy as np

P = 128
NSEQ = 512
D = 512
H = 8
DH = 64
NBANDS = 7
NCORES = 8
SCALE = D ** -0.5

_cached = None


def _emit_qkv(ctx, xt):
    """QKV projections for one band; returns (qk_sb, v_aug)."""
    nc, f32, f32r, bf16 = ctx["nc"], ctx["f32"], ctx["f32r"], ctx["bf16"]
    wq_sb = ctx["wq_sb"]
    pl = ctx["pools"]

    # q,k -> qkvT layout; q and k in SEPARATE tiles: the S matmuls read
    # lhsT from k and rhs from q, and same-tile operands measurably
    # slow the PE (SBUF port contention)
    q_sb = pl["qk"].tile([P, 4, NSEQ], ctx["qkdt"], tag="qk", name="q_sb")
    k_sb = pl["qk"].tile([P, 4, NSEQ], ctx["qkdt"], tag="qk", name="k_sb")
    for et in (0, 4, 1, 5, 2, 6, 3, 7):
        ps = pl["psproj"].tile([P, NSEQ], f32, tag="psproj", name="ps_qk")
        for kt in range(4):
            nc.tensor.matmul(
                ps[:], wq_sb[:, kt, et * P:(et + 1) * P], xt[:, kt, :],
                start=(kt == 0), stop=(kt == 3))
        dst = q_sb if et < 4 else k_sb
        nc.vector.tensor_copy(dst[:, et % 4, :], ps[:])

    # V projection -> row-major bf16 v_aug with ones column
    v_aug = pl["v"].tile([P, 4, H, DH + 1], bf16, tag="vaug", name="v_aug")
    for nt in range(4):
        ps = pl["psproj"].tile([P, NSEQ], f32, tag="psproj", name="ps_v")
        for kt in range(4):
            nc.tensor.matmul(
                ps[:], xt[:, kt, nt * P:(nt + 1) * P],
                wq_sb[:, kt, 2 * D:3 * D],
                start=(kt == 0), stop=(kt == 3))
        nc.vector.tensor_copy(
            v_aug[:, nt, :, 0:DH],
            ps[:].rearrange("p (h dh) -> p h dh", h=H))
        nc.vector.memset(v_aug[:, nt, :, DH:DH + 1], 1.0)
    return (q_sb, k_sb), v_aug


def _emit_attn(ctx, qk, v_aug):
    """S -> exp -> row-major AV -> per-partition normalize. Returns the
    four row-major O tiles (one per i-chunk)."""
    q_sb, k_sb = qk
    nc, f32, bf16, Exp = ctx["nc"], ctx["f32"], ctx["bf16"], ctx["Exp"]
    pl = ctx["pools"]

    es_store = {}
    o_sb = {}

    def s_phase(g):
        es_list = []
        for jt in range(4):
            # both head-halves in one 2-bank psum tile -> single exp
            ps_s = pl["pss"].tile([P, 2, NSEQ], f32, tag="pss", name="ps_s")
            nc.tensor.matmul(
                ps_s[:, 0, :],
                k_sb[0:DH, g, jt * P:(jt + 1) * P],
                q_sb[0:DH, g, :], start=True, stop=True)
            nc.tensor.matmul(
                ps_s[:, 1, :],
                k_sb[DH:P, g, jt * P:(jt + 1) * P],
                q_sb[DH:P, g, :], start=True, stop=True,
                tile_position=(DH, 0))
            es = pl["es"].tile([P, 2, NSEQ], bf16, tag="es", name="es")
            nc.scalar.activation(es[:], ps_s[:], Exp, scale=SCALE)
            es_list.append(es)
        es_store[g] = es_list

    def av_phase(g):
        es_list = es_store.pop(g)
        for ic in range(4):
            if g == 0:
                o_sb[ic] = pl["o"].tile([P, D], bf16, tag="osb",
                                        name=f"osb{ic}")
            if ctx["ablate"] == "no_av":
                if g == 0:
                    nc.vector.tensor_copy(o_sb[ic][:], es_list[0][:, 0, :])
                continue
            ps_av = pl["psav"].tile([P, 2, DH + 1], f32, tag="psav",
                                    name="ps_av")
            for h01 in range(2):
                for jt in range(4):
                    nc.tensor.matmul(
                        ps_av[:, h01, :],
                        es_list[jt][:, h01, ic * P:(ic + 1) * P],
                        v_aug[:, jt, 2 * g + h01, :],
                        start=(jt == 0), stop=(jt == 3))
            rcp = pl["r"].tile([P, 2], f32, tag="rcp", name="rcp")
            nc.vector.reciprocal(rcp[:], ps_av[:, :, DH])
            for h01 in range(2):
                hd = 2 * g + h01
                nc.vector.tensor_scalar_mul(
                    o_sb[ic][:, hd * DH:(hd + 1) * DH],
                    ps_av[:, h01, 0:DH],
                    rcp[:, h01:h01 + 1])

    s_phase(0)
    s_phase(1)
    av_phase(0)
    s_phase(2)
    av_phase(1)
    s_phase(3)
    av_phase(2)
    av_phase(3)
    return o_sb


def _emit_transposes(ctx, o_sb):
    """O row-major -> O^T via PE transposes; returns ot_sb."""
    nc, bf16 = ctx["nc"], ctx["bf16"]
    ident = ctx["ident"]
    pl = ctx["pools"]

    ot_sb = pl["ot"].tile([P, 4, NSEQ], bf16, tag="ot", name="ot_sb")
    for ec in range(4):
        ps_t = pl["psav"].tile([P, NSEQ], bf16, tag="psav", name="ps_t")
        for ic in range(4):
            nc.tensor.transpose(
                ps_t[:, ic * P:(ic + 1) * P],
                o_sb[ic][:, ec * P:(ec + 1) * P],
                ident[:])
        if ctx.get("tcopy") == "act":
            nc.scalar.copy(ot_sb[:, ec, :], ps_t[:])
        else:
            nc.vector.tensor_copy(ot_sb[:, ec, :], ps_t[:])
    return ot_sb


def _emit_outproj(ctx, s, ot_sb):
    nc, f32 = ctx["nc"], ctx["f32"]
    wo_sb, bias_sb, out = ctx["wo_sb"], ctx["bias_sb"], ctx["out"]
    pl = ctx["pools"]

    for nt in range(4):
        ps = pl["psproj"].tile([P, NSEQ], f32, tag="psproj", name="ps_o")
        for kt in range(4):
            nc.tensor.matmul(
                ps[:], ot_sb[:, kt, nt * P:(nt + 1) * P], wo_sb[:, kt, :],
                start=(kt == 0), stop=(kt == 3))
        ob = pl["ob"].tile([P, D], f32, tag="ob", name="ob")
        nc.vector.tensor_add(ob[:], ps[:], bias_sb[:])
        nc.sync.dma_start(
            out[s].rearrange("(no ni) e -> ni no e", ni=P)[:, nt, :], ob[:])


def build_kernel(nbands=NBANDS, repeat=1, ablate="", sdt="bf16",
                 qkvdt="bf16", tcopy="dve"):
    import concourse.mybir as mybir
    import concourse.tile as tile
    from concourse import bacc

    f32 = mybir.dt.float32
    f32r = mybir.dt.float32r
    bf16 = mybir.dt.bfloat16
    Exp = mybir.ActivationFunctionType.Exp

    nc = bacc.Bacc("TRN2", target_bir_lowering=False, debug=False,
                   num_devices=NCORES)

    xdt = f32r if qkvdt == "f32r" else bf16
    xT = nc.dram_tensor("xT", [nbands, D, NSEQ], xdt, kind="ExternalInput").ap()
    wqkvT = nc.dram_tensor("wqkvT", [D, 3 * D], xdt, kind="ExternalInput").ap()
    woutT = nc.dram_tensor("woutT", [D, D], bf16, kind="ExternalInput").ap()
    biasb = nc.dram_tensor("biasb", [P, D], f32, kind="ExternalInput").ap()
    identT = nc.dram_tensor("identT", [P, P], bf16, kind="ExternalInput").ap()
    out = nc.dram_tensor("out", [nbands, NSEQ, D], f32, kind="ExternalOutput").ap()

    with tile.TileContext(nc) as tc:
        with (
            tc.tile_pool(name="weights", bufs=1) as wpool,
            tc.tile_pool(name="x", bufs=3) as xpool,
            tc.tile_pool(name="qk", bufs=2) as qkpool,
            tc.tile_pool(name="v", bufs=2) as vpool,
            tc.tile_pool(name="o", bufs=8) as opool,
            tc.tile_pool(name="ot", bufs=2) as otpool,
            tc.tile_pool(name="es", bufs=8) as spool,
            tc.tile_pool(name="r", bufs=4) as rpool,
            tc.tile_pool(name="ob", bufs=3) as outpool,
            tc.tile_pool(name="psproj", bufs=2, space="PSUM") as psproj,
            tc.tile_pool(name="pss", bufs=2, space="PSUM") as pss,
            tc.tile_pool(name="psav", bufs=2, space="PSUM") as psav,
        ):
            # weights: split wq by k-chunk so the first matmuls can start
            # as soon as their chunk lands
            wq_sb = wpool.tile([P, 4, 3 * D], xdt)
            wo_sb = wpool.tile([P, 4, D], bf16)
            bias_sb = wpool.tile([P, D], f32)
            ident = wpool.tile([P, P], bf16)
            wq_r = wqkvT.rearrange("(ko ki) e -> ki ko e", ki=P)
            for kt in range(4):
                nc.sync.dma_start(wq_sb[:, kt, :], wq_r[:, kt, :])
            nc.sync.dma_start(wo_sb[:], woutT.rearrange("(ko ki) e -> ki ko e", ki=P))
            nc.sync.dma_start(bias_sb[:], biasb[:])
            nc.sync.dma_start(ident[:], identT[:])

            ctx = {
                "nc": nc, "ablate": ablate, "tcopy": tcopy,
                "qkdt": f32r if sdt == "f32r" else bf16,
                "f32": f32, "f32r": f32r, "bf16": bf16, "Exp": Exp,
                "wq_sb": wq_sb, "wo_sb": wo_sb, "bias_sb": bias_sb,
                "ident": ident,
                "out": out,
                "pools": {
                    "qk": qkpool, "v": vpool, "o": opool, "ot": otpool,
                    "es": spool, "r": rpool, "ob": outpool,
                    "psproj": psproj, "pss": pss, "psav": psav,
                },
            }

            def load_x(s):
                xt = xpool.tile([P, 4, NSEQ], xdt, tag="xt")
                nc.sync.dma_start(
                    xt[:], xT[s].rearrange("(ko ki) n -> ki ko n", ki=P))
                return xt

            rep_ctx = (tc.For_i(0, repeat, 1,
                                hint_engines=(mybir.EngineType.PE,
                                              mybir.EngineType.Activation,
                                              mybir.EngineType.DVE))
                       if repeat > 1 else contextlib.nullcontext())
            with rep_ctx:
                # software pipeline: attn(s) -> transposes(s) -> QKV(s+1)
                # -> out-proj(s), so transpose copies and out-proj deps
                # hide under next-band QKV matmuls.
                xt = load_x(0)
                qkv = _emit_qkv(ctx, xt)
                for s in range(nbands):
                    if s + 1 < nbands:
                        xt_next = load_x(s + 1)
                    if ctx["ablate"] == "no_attn":
                        o_sb = {}
                        for ic in range(4):
                            o_sb[ic] = opool.tile([P, D], bf16, tag="osb",
                                                  name=f"osb{ic}")
                            nc.vector.tensor_copy(
                                o_sb[ic][:].rearrange(
                                    "p (h dh) -> p h dh", h=H),
                                qkv[1][:, ic, :, 0:DH])
                    else:
                        o_sb = _emit_attn(ctx, *qkv)
                    ot_sb = _emit_transposes(ctx, o_sb)
                    if s + 1 < nbands:
                        qkv = _emit_qkv(ctx, xt_next)
                    _emit_outproj(ctx, s, ot_sb)

    nc.compile()
    return nc


def _get_nc():
    global _cached
    if _cached is None:
        _cached = build_kernel()
    return _cached


def make_in_maps(x, x_delta, x_theta, x_alpha, x_beta, x_gamma, x_upper,
                 Wqkv, Wout, bout, qkvdt="bf16"):
    import ml_dtypes
    bf16 = ml_dtypes.bfloat16
    xdt = np.float32 if qkvdt == "f32r" else bf16
    xs = np.stack([np.asarray(a, dtype=np.float32) for a in
                   (x, x_delta, x_theta, x_alpha, x_beta, x_gamma, x_upper)],
                  axis=0)  # [7, b, n, d]
    xsT = np.ascontiguousarray(xs.transpose(1, 0, 3, 2).astype(xdt))
    wqkvT = np.ascontiguousarray(np.asarray(Wqkv, np.float32).T.astype(xdt))
    woutT = np.ascontiguousarray(np.asarray(Wout, np.float32).T.astype(bf16))
    biasb = np.ascontiguousarray(
        np.broadcast_to(np.asarray(bout, np.float32)[None, :], (P, D)))
    identT = np.eye(P, dtype=bf16)
    return [
        {"xT": xsT[c], "wqkvT": wqkvT, "woutT": woutT, "biasb": biasb,
         "identT": identT}
        for c in range(NCORES)
    ]


def kernel(x, x_delta, x_theta, x_alpha, x_beta, x_gamma, x_upper,
           Wqkv, Wout, bout):
    from concourse.bass_utils import run_bass_kernel_spmd

    nc = _get_nc()
    in_maps = make_in_maps(x, x_delta, x_theta, x_alpha, x_beta, x_gamma,
                           x_upper, Wqkv, Wout, bout)
    res = run_bass_kernel_spmd(nc, in_maps, core_ids=list(range(NCORES)))
    full = np.empty((NBANDS, NCORES, NSEQ, D), dtype=np.float32)
    for c in range(NCORES):
        full[:, c] = res.results[c]["out"]
    return tuple(full[i] for i in range(NBANDS))
